# revision 10
# baseline (speedup 1.0000x reference)
"""Trainium2 Bass kernel for LCGNoiseLayer2D: relu(x+noise) -> 1x1 conv -> BatchNorm2d.

Sharding: data-parallel over batch (B=32 -> 4 per core x 8 cores); noise/weights
replicated; BN batch statistics all-reduced across cores (tiny 3KB collective).

Math:  yhat = W @ relu(x + noise)            (conv bias b cancels inside BN)
       mean_o = E[yhat_o] + b ; var_o = E[yhat_o^2] - E[yhat_o]^2  (bias-invariant)
       out = yhat * scale + bias_f,  scale = gamma*rsqrt(var+eps),
                                     bias_f = beta - E[yhat]*scale
Pass 1 computes per-core sums s_c = sum_p relu(x+noise)[c,p] (free with the relu
on ACT via accum_out) and ssq_o = sum_p yhat_o^2 (Square/mult-reduce on the PSUM
tiles).  One 8-core AllReduce of [128,6] f32.  Pass 2 re-runs the matmul and
applies the per-channel affine on the PSUM->SBUF copy (alternating DVE/ACT).
"""

import numpy as np

import concourse.bass as bass
import concourse.mybir as mybir
import concourse.tile as tile
from concourse import bacc
from concourse.bass_utils import run_bass_kernel_spmd

# Problem shapes (hardcoded per contract)
B, CIN, COUT, H, W = 32, 256, 512, 64, 64
HW = H * W                      # 4096
NCORES = 8
NB = B // NCORES                # 4 batches per core
P = 128
CC = CIN // P                   # 2 c-chunks
OC = COUT // P                  # 4 o-chunks
PT = 512                        # pixels per psum tile
PPB = HW // PT                  # 8 psum tiles per (o-chunk, batch)
NTOT = float(B * HW)            # global pixel count for BN stats
BN_EPS = 1e-5

LEVEL = 0.1
SEED = 12345
RAND_MAX = 4294967295.0

FP = mybir.dt.float32
ALU = mybir.AluOpType
ACTF = mybir.ActivationFunctionType


def _lcg_noise() -> np.ndarray:
    """Exact uint32 LCG jump-ahead, matching the jax reference bit-for-bit."""
    n = CIN * HW
    m = np.full((n,), 65539, dtype=np.uint32)
    with np.errstate(over="ignore"):
        powers = np.cumprod(m, dtype=np.uint32)
        geom = np.cumsum(
            np.concatenate([np.ones((1,), np.uint32), powers[:-1]]), dtype=np.uint32
        )
        seeds = (powers * np.uint32(SEED) + geom).astype(np.uint32)
    return (seeds.astype(np.float32) * np.float32(LEVEL / RAND_MAX)).reshape(CIN, HW)


def build_nc():
    nc = bacc.Bacc(None, target_bir_lowering=False, num_devices=NCORES)

    x_h = nc.dram_tensor("x", [NB, CIN, HW], FP, kind="ExternalInput")
    noise_h = nc.dram_tensor("noise", [CIN, HW], FP, kind="ExternalInput")
    wt_h = nc.dram_tensor("wt", [CIN, COUT], FP, kind="ExternalInput")
    gamma_h = nc.dram_tensor("gamma", [P, OC], FP, kind="ExternalInput")
    beta_h = nc.dram_tensor("beta", [P, OC], FP, kind="ExternalInput")
    out_h = nc.dram_tensor("out", [NB, COUT, HW], FP, kind="ExternalOutput")

    with tile.TileContext(nc) as tc:
        with (
            tc.tile_pool(name="xp", bufs=CC * NB) as xp,
            tc.tile_pool(name="np_", bufs=CC) as np_,
            tc.tile_pool(name="wp", bufs=1) as wp,
            tc.tile_pool(name="cst", bufs=1) as cst,
            tc.tile_pool(name="sqp", bufs=3) as sqp,
            tc.tile_pool(name="outp", bufs=4) as outp,
            tc.tile_pool(name="psum", bufs=8, space="PSUM") as psum,
            tc.tile_pool(name="dram", bufs=1, space="DRAM") as dram,
        ):
            # ---- constants / weights ----
            wt_sb = wp.tile([P, CC, COUT], FP)
            for cc in range(CC):
                nc.sync.dma_start(wt_sb[:, cc, :], wt_h[cc * P : (cc + 1) * P, :])
            gamma_col = cst.tile([P, OC], FP, tag="gcol")
            beta_col = cst.tile([P, OC], FP, tag="bcol")
            nc.sync.dma_start(gamma_col[:], gamma_h[:])
            nc.sync.dma_start(beta_col[:], beta_h[:])

            # ---- phase 1: load x, x2 = relu(x + noise), s_c = sum(x2) ----
            s_acc = cst.tile([P, CC * NB], FP, tag="sacc")
            noise_t = {}
            for cc in range(CC):
                nt = np_.tile([P, HW], FP, tag="noise")
                nc.sync.dma_start(nt[:], noise_h[cc * P : (cc + 1) * P, :])
                noise_t[cc] = nt
            x2 = {}
            for cc in range(CC):
                for b in range(NB):
                    xt = xp.tile([P, HW], FP, tag="x2")
                    nc.sync.dma_start(xt[:], x_h[b, cc * P : (cc + 1) * P, :])
                    nc.vector.tensor_tensor(xt[:], xt[:], noise_t[cc][:], ALU.add)
                    nc.scalar.activation(
                        xt[:], xt[:], ACTF.Relu,
                        accum_out=s_acc[:, cc * NB + b : cc * NB + b + 1],
                    )
                    x2[(cc, b)] = xt

            s_red = cst.tile([P, CC], FP, tag="sred")
            for cc in range(CC):
                nc.vector.tensor_reduce(
                    s_red[:, cc : cc + 1], s_acc[:, cc * NB : (cc + 1) * NB],
                    mybir.AxisListType.X, ALU.add,
                )

            # ---- phase 2: pass-1 matmul, ssq_o = sum(yhat^2) ----
            ssq_acc = cst.tile([P, OC, NB * PPB], FP, tag="qacc")
            tix = 0
            for oc in range(OC):
                for b in range(NB):
                    for ptg in range(0, PPB, 4):
                        tiles = []
                        for i in range(4):
                            ps = psum.tile([P, PT], FP, tag="ps")
                            tiles.append((ps, ptg + i))
                        for cc in range(CC):
                            for ps, pt in tiles:
                                nc.tensor.matmul(
                                    ps[:],
                                    wt_sb[:, cc, oc * P : (oc + 1) * P],
                                    x2[(cc, b)][:, pt * PT : (pt + 1) * PT],
                                    start=(cc == 0), stop=(cc == CC - 1),
                                )
                        for ps, pt in tiles:
                            sq = sqp.tile([P, PT], FP, tag="sq")
                            col = b * PPB + pt
                            nc.scalar.activation(
                                sq[:], ps[:], ACTF.Square,
                                accum_out=ssq_acc[:, oc, col : col + 1],
                            )

            ssq_red = cst.tile([P, OC], FP, tag="qred")
            for oc in range(OC):
                nc.vector.tensor_reduce(
                    ssq_red[:, oc : oc + 1], ssq_acc[:, oc, :],
                    mybir.AxisListType.X, ALU.add,
                )

            # ---- phase 3: all-reduce stats, fold scale/bias ----
            stat_sb = cst.tile([P, CC + OC], FP, tag="statsb")
            nc.vector.tensor_copy(stat_sb[:, 0:CC], s_red[:])
            nc.vector.tensor_copy(stat_sb[:, CC : CC + OC], ssq_red[:])
            bounce_in = dram.tile([P, CC + OC], FP, tag="bin")
            bounce_out = dram.tile([P, CC + OC], FP, tag="bout")
            nc.gpsimd.dma_start(bounce_in[:], stat_sb[:])
            nc.gpsimd.collective_compute(
                "AllReduce", ALU.add,
                replica_groups=[list(range(NCORES))],
                ins=[bounce_in.opt()],
                outs=[bounce_out.opt()],
            )
            stot = cst.tile([P, CC + OC], FP, tag="stot")
            nc.sync.dma_start(stot[:], bounce_out[:])

            # sum_c W[o,c] * s_tot[c] per o-chunk -> [128, OC] column layout (PSUM)
            mu_col = cst.tile([P, OC], FP, tag="mucol")
            for oc in range(OC):
                sw_ps = psum.tile([P, 1], FP, tag="ps")
                for cc in range(CC):
                    nc.tensor.matmul(
                        sw_ps[:], wt_sb[:, cc, oc * P : (oc + 1) * P],
                        stot[:, cc : cc + 1],
                        start=(cc == 0), stop=(cc == CC - 1),
                    )
                nc.vector.tensor_copy(mu_col[:, oc : oc + 1], sw_ps[:])
            # mu = (W s)/N ; e2 = ssq/N ; var = e2 - mu^2 ; all in [128, OC] col layout
            nc.vector.tensor_scalar_mul(mu_col[:], mu_col[:], 1.0 / NTOT)
            e2_col = cst.tile([P, OC], FP, tag="e2col")
            nc.vector.tensor_scalar_mul(e2_col[:], stot[:, CC : CC + OC], 1.0 / NTOT)
            msq_col = cst.tile([P, OC], FP, tag="msqcol")
            nc.vector.tensor_tensor(msq_col[:], mu_col[:], mu_col[:], ALU.mult)
            var_col = cst.tile([P, OC], FP, tag="varcol")
            nc.vector.tensor_tensor(var_col[:], e2_col[:], msq_col[:], ALU.subtract)
            sd_col = cst.tile([P, OC], FP, tag="sdcol")
            eps_col = cst.tile([P, 1], FP, tag="epscol")
            nc.vector.memset(eps_col[:], BN_EPS)
            nc.scalar.activation(sd_col[:], var_col[:], ACTF.Sqrt, bias=eps_col[:])
            rstd_col = cst.tile([P, OC], FP, tag="rstdcol")
            nc.vector.reciprocal(rstd_col[:], sd_col[:])
            scale_col = cst.tile([P, OC], FP, tag="scol")
            nc.vector.tensor_tensor(scale_col[:], rstd_col[:], gamma_col[:], ALU.mult)
            bias_col = cst.tile([P, OC], FP, tag="bfcol")
            nc.vector.tensor_tensor(bias_col[:], mu_col[:], scale_col[:], ALU.mult)
            nc.vector.tensor_tensor(bias_col[:], beta_col[:], bias_col[:], ALU.subtract)

            # ---- phase 4: pass-2 matmul, affine on PSUM->SBUF copy, DMA out ----
            tix = 0
            for oc in range(OC):
                for b in range(NB):
                    for ptg in range(0, PPB, 4):
                        tiles = []
                        for i in range(4):
                            ps = psum.tile([P, PT], FP, tag="ps")
                            tiles.append((ps, ptg + i))
                        for cc in range(CC):
                            for ps, pt in tiles:
                                nc.tensor.matmul(
                                    ps[:],
                                    wt_sb[:, cc, oc * P : (oc + 1) * P],
                                    x2[(cc, b)][:, pt * PT : (pt + 1) * PT],
                                    start=(cc == 0), stop=(cc == CC - 1),
                                )
                        for ps, pt in tiles:
                            ot = outp.tile([P, PT], FP, tag="ot")
                            nc.vector.tensor_scalar(
                                ot[:], ps[:],
                                scale_col[:, oc : oc + 1],
                                bias_col[:, oc : oc + 1],
                                ALU.mult, ALU.add,
                            )
                            nc.sync.dma_start(
                                out_h[b, oc * P : (oc + 1) * P, pt * PT : (pt + 1) * PT],
                                ot[:],
                            )

    nc.compile()
    return nc


_NC_CACHE = None


def kernel(**inputs) -> np.ndarray:
    global _NC_CACHE
    x = np.ascontiguousarray(inputs["x"], dtype=np.float32)
    conv_w = np.asarray(inputs["conv_w"], dtype=np.float32)
    gamma = np.ascontiguousarray(inputs["gamma"], dtype=np.float32)
    beta = np.ascontiguousarray(inputs["beta"], dtype=np.float32)
    # conv_b cancels inside BatchNorm (shift-invariance); unused by design.

    noise = _lcg_noise()
    wt = np.ascontiguousarray(conv_w.T)  # [CIN, COUT]

    if _NC_CACHE is None:
        _NC_CACHE = build_nc()
    nc = _NC_CACHE

    xs = x.reshape(B, CIN, HW)
    in_maps = []
    for c in range(NCORES):
        in_maps.append({
            "x": np.ascontiguousarray(xs[c * NB : (c + 1) * NB]),
            "noise": noise,
            "wt": wt,
            "gamma": np.ascontiguousarray(gamma.reshape(OC, P).T),
            "beta": np.ascontiguousarray(beta.reshape(OC, P).T),
        })
    res = run_bass_kernel_spmd(nc, in_maps, core_ids=list(range(NCORES)))
    out = np.concatenate([res.results[c]["out"] for c in range(NCORES)], axis=0)
    return out.reshape(B, COUT, H, W)
